# revision 9
# baseline (speedup 1.0000x reference)
"""Canny edge detection (NMS mask) Bass kernel for trn2, 8-core data parallel.

Key reductions vs the reference:
  - Z (the NMS mask) IS the final output: weak = (Z>=0.2)&(Z<=0.6) is always
    false for Z in {0,1}, so hysteresis/dilation is a no-op.
  - mag comparisons done on squared magnitudes (sqrt is monotone).
  - angle binning via tangent-squared ratio comparisons + sign(gx*gy):
      m0   : t1^2*gx^2 >  gy^2
      m90  : t2^2*gx^2 <= gy^2
      diag : else; m45 if gx*gy>=0 else m135      (t1=tan22.5, t2=tan67.5)
  - separable convs: vertical factors as banded-matrix matmuls on PE
    (zero-pad boundary handled by natural band truncation), horizontal
    factors as shifted-AP fused DVE ops.
"""

import numpy as np

H = W = 512
B = 32
N_CORES = 8
IMGS_PER_CORE = B // N_CORES
# chunk starts; each chunk covers rows [s, s+128); valid Z rows [s+4, s+124)
# except first/last chunk which extend to the image boundary.
CHUNK_STARTS = [0, 120, 240, 360, 384]
# emitted Z rows (absolute) per chunk (non-overlapping cover of [0,512))
CHUNK_OUT = [(0, 124), (124, 244), (244, 364), (364, 484), (484, 512)]

_GRAY_W = (0.299, 0.587, 0.114)


def _g1n():
    ax = np.arange(-2, 3, dtype=np.float64)
    g = np.exp(-(ax * ax) / 2.0)
    return g / g.sum()


def _band(w, off):
    """lhsT[k, m] = w[d] where k = m + d - off, k clipped to [0,128).

    out[m] = sum_d w[d] * in[m + d - off]; out = lhsT.T @ in on PE.
    """
    Bm = np.zeros((128, 128), np.float32)
    idx = np.arange(128)
    for d, wv in enumerate(w):
        kk = idx + d - off
        valid = (kk >= 0) & (kk < 128)
        Bm[kk[valid], idx[valid]] = np.float32(wv)
    return Bm


def _weights():
    g1 = _g1n()
    bands = []
    for wch in _GRAY_W:
        bands.append(_band(wch * g1, 2))        # 0,1,2: gray+gaussV per channel
    bands.append(_band([1.0, 2.0, 1.0], 1))     # 3: Sv (for gx)
    bands.append(_band([-1.0, 0.0, 1.0], 1))    # 4: Dv (for gy)
    bands.append(_band([1.0], 1))               # 5: up   u[m] = in[m-1]
    bands.append(_band([1.0], -1))              # 6: down d[m] = in[m+1]
    return np.stack(bands).astype(np.float32)   # [7,128,128]


_NC_CACHE = {}


def _build(n_reps):
    import concourse.bacc as bacc
    import concourse.tile as tile
    from concourse import mybir

    f32 = mybir.dt.float32
    Alu = mybir.AluOpType

    g1 = _g1n()
    a_over_b = float(np.float32(g1[0] / g1[1]))
    b_over_c = float(np.float32(g1[1] / g1[2]))
    t1sq = float(np.float32(np.tan(np.deg2rad(22.5)) ** 2))
    t2sq = float(np.float32(np.tan(np.deg2rad(67.5)) ** 2))

    nc = bacc.Bacc("TRN2", target_bir_lowering=False, debug=False,
                   num_devices=N_CORES)
    x_d = nc.dram_tensor("x", [IMGS_PER_CORE, 3, H, W], f32,
                         kind="ExternalInput").ap()
    wb_d = nc.dram_tensor("wb", [7, 128, 128], f32, kind="ExternalInput").ap()
    y_d = nc.dram_tensor("y", [IMGS_PER_CORE, 1, H, W], f32,
                         kind="ExternalOutput").ap()

    with tile.TileContext(nc) as tc:
        import contextlib
        with contextlib.ExitStack() as ctx:
            wpool = ctx.enter_context(tc.tile_pool(name="w", bufs=1))
            xpool = ctx.enter_context(tc.tile_pool(name="xin", bufs=2))
            sb = ctx.enter_context(tc.tile_pool(name="sb", bufs=2))
            psA = ctx.enter_context(tc.tile_pool(name="psA", bufs=2, space="PSUM"))
            psO = ctx.enter_context(tc.tile_pool(name="psO", bufs=1, space="PSUM"))

            wt = wpool.tile([128, 7 * 128], f32)
            nc.sync.dma_start(
                wt[:].rearrange("k (n m) -> k n m", n=7),
                wb_d.rearrange("n k m -> k n m"))

            def wslice(i):
                return wt[:, i * 128:(i + 1) * 128]

            import contextlib as _ctl

            rep_ctx = (tc.For_i(0, n_reps, 1) if n_reps > 1
                       else _ctl.nullcontext())
            with rep_ctx:
                for img in range(IMGS_PER_CORE):
                    for ci, s in enumerate(CHUNK_STARTS):
                        r0, r1 = CHUNK_OUT[ci]
                        # ---- load x chunk: [128 rows, 3ch, 512] one DMA
                        xt = xpool.tile([128, 3 * W], f32, tag="xt")
                        nc.sync.dma_start(
                            xt[:].rearrange("p (c w) -> p c w", c=3),
                            x_d[img].rearrange("c h w -> h c w")[s:s + 128],
                        )
                        # ---- A = sum_ch (gray_w*gaussV band) @ x_ch  (PE)
                        A_ps = psA.tile([128, W], f32, tag="A")
                        for ch in range(3):
                            nc.tensor.matmul(
                                A_ps[:], wslice(ch),
                                xt[:, ch * W:(ch + 1) * W],
                                start=(ch == 0), stop=(ch == 2),
                            )
                        # ---- A -> guarded sbuf tile [128, 516] (ACT copy)
                        Asb = sb.tile([128, W + 4], f32, tag="Asb")
                        nc.gpsimd.memset(Asb[:, 0:2], 0.0)
                        nc.gpsimd.memset(Asb[:, W + 2:W + 4], 0.0)
                        nc.scalar.copy(Asb[:, 2:W + 2], A_ps[:])
                        # ---- blurH: 5-tap symmetric, scale-free chain (DVE)
                        p1 = sb.tile([128, W], f32, tag="p1")
                        nc.vector.tensor_add(p1[:], Asb[:, 0:W], Asb[:, 4:W + 4])
                        p2 = sb.tile([128, W], f32, tag="p2")
                        nc.vector.tensor_add(p2[:], Asb[:, 1:W + 1], Asb[:, 3:W + 3])
                        r1t = sb.tile([128, W], f32, tag="r1t")
                        nc.vector.scalar_tensor_tensor(
                            r1t[:], p1[:], a_over_b, p2[:],
                            op0=Alu.mult, op1=Alu.add)
                        blur = sb.tile([128, W + 2], f32, tag="blur")
                        nc.gpsimd.memset(blur[:, 0:1], 0.0)
                        nc.gpsimd.memset(blur[:, W + 1:W + 2], 0.0)
                        nc.vector.scalar_tensor_tensor(
                            blur[:, 1:W + 1], r1t[:], b_over_c, Asb[:, 2:W + 2],
                            op0=Alu.mult, op1=Alu.add)
                        # ---- Dh and Sh (DVE)
                        dxh = sb.tile([128, W], f32, tag="dxh")
                        nc.vector.tensor_tensor(
                            dxh[:], blur[:, 2:W + 2], blur[:, 0:W],
                            op=Alu.subtract)
                        shp = sb.tile([128, W], f32, tag="shp")
                        nc.vector.tensor_add(shp[:], blur[:, 0:W], blur[:, 2:W + 2])
                        sh = sb.tile([128, W], f32, tag="sh")
                        nc.vector.scalar_tensor_tensor(
                            sh[:], blur[:, 1:W + 1], 2.0, shp[:],
                            op0=Alu.mult, op1=Alu.add)
                        # ---- gx, gy (PE)
                        gx_ps = psO.tile([128, W], f32, tag="gx")
                        nc.tensor.matmul(gx_ps[:], wslice(3), dxh[:],
                                         start=True, stop=True)
                        gy_ps = psO.tile([128, W], f32, tag="gy")
                        nc.tensor.matmul(gy_ps[:], wslice(4), sh[:],
                                         start=True, stop=True)
                        # ---- squares (ACT), msq, pxy (DVE)
                        sqx = sb.tile([128, W], f32, tag="sqx")
                        nc.scalar.square(sqx[:], gx_ps[:])
                        gysb = sb.tile([128, W], f32, tag="gysb")
                        nc.scalar.copy(gysb[:], gy_ps[:])
                        sqy = sb.tile([128, W], f32, tag="sqy")
                        nc.scalar.square(sqy[:], gysb[:])
                        msq = sb.tile([128, W + 2], f32, tag="msq")
                        nc.gpsimd.memset(msq[:, 0:1], 0.0)
                        nc.gpsimd.memset(msq[:, W + 1:W + 2], 0.0)
                        nc.vector.tensor_add(msq[:, 1:W + 1], sqx[:], sqy[:])
                        pxy = sb.tile([128, W], f32, tag="pxy")
                        nc.vector.tensor_mul(pxy[:], gx_ps[:], gysb[:])
                        # ---- bin masks (DVE)
                        m0 = sb.tile([128, W], mybir.dt.uint8, tag="m0")
                        nc.vector.scalar_tensor_tensor(
                            m0[:], sqx[:], t1sq, sqy[:],
                            op0=Alu.mult, op1=Alu.is_gt)
                        m90 = sb.tile([128, W], mybir.dt.uint8, tag="m90")
                        nc.vector.scalar_tensor_tensor(
                            m90[:], sqx[:], t2sq, sqy[:],
                            op0=Alu.mult, op1=Alu.is_le)
                        s45 = sb.tile([128, W], mybir.dt.uint8, tag="s45")
                        nc.vector.tensor_scalar(
                            s45[:], pxy[:], 0.0, None, op0=Alu.is_ge)
                        # ---- u/d row shifts of msq (PE) + guarded copies (ACT)
                        u_ps = psO.tile([128, W], f32, tag="u")
                        nc.tensor.matmul(u_ps[:], wslice(5), msq[:, 1:W + 1],
                                         start=True, stop=True)
                        d_ps = psO.tile([128, W], f32, tag="d")
                        nc.tensor.matmul(d_ps[:], wslice(6), msq[:, 1:W + 1],
                                         start=True, stop=True)
                        usb = sb.tile([128, W + 2], f32, tag="usb")
                        nc.gpsimd.memset(usb[:, 0:1], 0.0)
                        nc.gpsimd.memset(usb[:, W + 1:W + 2], 0.0)
                        nc.scalar.copy(usb[:, 1:W + 1], u_ps[:])
                        dsb = sb.tile([128, W + 2], f32, tag="dsb")
                        nc.gpsimd.memset(dsb[:, 0:1], 0.0)
                        nc.gpsimd.memset(dsb[:, W + 1:W + 2], 0.0)
                        nc.scalar.copy(dsb[:, 1:W + 1], d_ps[:])
                        # ---- neighbor maxes + predicated cascade (DVE)
                        msel = sb.tile([128, W], f32, tag="msel")
                        # M135 = max(ul, dr) = max(u[j-1], d[j+1])
                        nc.vector.tensor_tensor(
                            msel[:], usb[:, 0:W], dsb[:, 2:W + 2], op=Alu.max)
                        m45t = sb.tile([128, W], f32, tag="m45t")
                        # M45 = max(dl, ur) = max(d[j-1], u[j+1])
                        nc.vector.tensor_tensor(
                            m45t[:], dsb[:, 0:W], usb[:, 2:W + 2], op=Alu.max)
                        nc.vector.copy_predicated(msel[:], s45[:], m45t[:])
                        m90t = sb.tile([128, W], f32, tag="m90t")
                        nc.vector.tensor_tensor(
                            m90t[:], usb[:, 1:W + 1], dsb[:, 1:W + 1], op=Alu.max)
                        nc.vector.copy_predicated(msel[:], m90[:], m90t[:])
                        m0t = sb.tile([128, W], f32, tag="m0t")
                        nc.vector.tensor_tensor(
                            m0t[:], msq[:, 0:W], msq[:, 2:W + 2], op=Alu.max)
                        nc.vector.copy_predicated(msel[:], m0[:], m0t[:])
                        # ---- Z = msq >= Msel
                        z = sb.tile([128, W], f32, tag="z")
                        nc.vector.tensor_tensor(
                            z[:], msq[:, 1:W + 1], msel[:], op=Alu.is_ge)
                        # ---- store valid rows
                        lr0, lr1 = r0 - s, r1 - s
                        nc.sync.dma_start(y_d[img, 0, r0:r1, :],
                                          z[lr0:lr1, :])
    nc.compile()
    return nc


def _get_nc(n_reps):
    if n_reps not in _NC_CACHE:
        _NC_CACHE[n_reps] = _build(n_reps)
    return _NC_CACHE[n_reps]


def run_on_cores(x, n_reps=1):
    from concourse.bass_utils import run_bass_kernel_spmd

    nc = _get_nc(n_reps)
    wb = _weights()
    x = np.ascontiguousarray(np.asarray(x), dtype=np.float32)
    in_maps = [
        {"x": x[c * IMGS_PER_CORE:(c + 1) * IMGS_PER_CORE], "wb": wb}
        for c in range(N_CORES)
    ]
    res = run_bass_kernel_spmd(nc, in_maps, list(range(N_CORES)))
    out = np.concatenate([res.results[c]["y"] for c in range(N_CORES)], axis=0)
    return out


def kernel(x):
    return run_on_cores(x, n_reps=1)

